# revision 5
# baseline (speedup 1.0000x reference)
"""CrossAttention TRN2 kernel (v3).

Problem (hardcoded shapes):
  x    [4, 2048, 1024], cond [4, 2048, 1024]
  Wq/Wk/Wv [1024, 1024], Wo [1024, 1024], bo [1024]
  out = softmax((x@Wq) 8 heads of 128 @ (cond@Wk)^T * 0.125) @ (cond@Wv) @ Wo + bo

Sharding: 8 cores = (batch b in 0..3) x (query-half ih in 0..1).
Each core: 1024 query rows of one batch, all 8 heads; K/V projection for
that batch replicated across the 2 cores sharing it. No collectives.

v3 design:
- Matmul inputs bf16 (x, cond, weights; cast on host); exp/V/denominator in
  fp16 (16-bit DVE/pool rate, no mixed-dtype ops); PSUM f32.
- x^T and cond^T fully resident in SBUF.
- Softmax denominator: gpsimd accumulates fp16 exp tiles (SBUF only), a
  ones-stationary fp16 matmul does the partition reduction into PSUM per
  (head, ih), DVE reciprocal (fp16 out), DVE high-priority normalize mul.
- Head group g's attention interleaves group g+1's projection matmuls into
  the tensor queue so the PE never waits on the scalar engine's exp.
- O-projection split-k: k=0..5 (heads 0..5) interleaved into group-3
  attention, k=6..7 + combine + bias at the end.
- av PSUM tiles bufs=4 and single-live projection accumulators, so head
  boundaries don't stall on PSUM bank reuse.
"""
import numpy as np
import ml_dtypes

import concourse.bass as bass
import concourse.bacc as bacc
import concourse.tile as tile
from concourse import bass_isa, mybir
from concourse.bass_utils import run_bass_kernel_spmd

F32 = mybir.dt.float32
F32R = mybir.dt.float32r
BF16 = mybir.dt.bfloat16
FP16 = mybir.dt.float16
EXP = mybir.ActivationFunctionType.Exp

B, NQ, NK, D = 4, 2048, 2048, 1024   # D = query_dim = cond_dim = inner_dim
H, DH = 8, 128                        # heads, per-head dim
SCALE = 64 ** -0.5                    # reference uses dim_head=64 for the scale
NCORES = 8
IQ = NQ // 2                          # query rows per core (1024)
KT = D // 128                         # contraction tiles (8)
GROUPS, HPG = 4, 2                    # head groups of 2 heads
JT = NK // 128                        # key tiles (16)
IH = IQ // 512                        # 512-wide query chunks (2)
OSPLIT = 6                            # O-proj k tiles interleaved into att(3)


def build_nc():
    nc = bacc.Bacc()
    xT = nc.declare_dram_parameter("xT", [D, IQ], BF16, isOutput=False)
    condT = nc.declare_dram_parameter("condT", [D, NK], BF16, isOutput=False)
    wq = nc.declare_dram_parameter("wq", [D, D], BF16, isOutput=False)
    wk = nc.declare_dram_parameter("wk", [D, D], BF16, isOutput=False)
    wv = nc.declare_dram_parameter("wv", [D, D], BF16, isOutput=False)
    wo = nc.declare_dram_parameter("wo", [D, D], BF16, isOutput=False)
    bo = nc.declare_dram_parameter("bo", [1, D], F32, isOutput=False)
    out = nc.declare_dram_parameter("out", [IQ, D], F32, isOutput=True)

    with tile.TileContext(nc) as tc:
        with (
            nc.allow_low_precision(reason="bf16/fp16 matmul operands intended"),
            tc.tile_pool(name="const", bufs=1) as const,
            tc.tile_pool(name="big", bufs=1) as big,
            tc.tile_pool(name="grp", bufs=2) as grp,
            tc.tile_pool(name="expp", bufs=4) as expp,
            tc.tile_pool(name="small", bufs=2) as small,
            tc.tile_pool(name="ostage", bufs=2) as ostage,
            tc.tile_pool(name="oacc", bufs=1) as oacc,
            tc.tile_pool(name="ps", bufs=1, space="PSUM") as ps,
        ):
            bo_bc = const.tile([128, D], F32)
            ones = const.tile([128, 128], FP16)
            ones_f = const.tile([128, 128], F32)

            xr = big.tile([128, KT, IQ], BF16)    # resident x^T
            ct = big.tile([128, KT, NK], BF16)    # resident cond^T
            attT = big.tile([128, H, IQ], BF16)   # attention out, [dh, head, i]
            wo_r = big.tile([128, KT, D], BF16)
            # O-proj partial sums (k=0..OSPLIT-1), staged in SBUF
            foa = oacc.tile([128, IQ // 128, D // 512, 512], F32)

            def load_group_weights(g, interleave=False):
                c0 = g * HPG * DH
                gw = HPG * DH
                wq_g = grp.tile([128, KT, gw], BF16, tag="wq_g", name=f"wq_{g}")
                wk_g = grp.tile([128, KT, gw], BF16, tag="wk_g", name=f"wk_{g}")
                wv_g = grp.tile([128, KT, gw], BF16, tag="wv_g", name=f"wv_{g}")
                for k in range(KT):
                    rows = slice(k * 128, (k + 1) * 128)
                    nc.sync.dma_start(out=wq_g[:, k, :], in_=wq[rows, c0:c0 + gw])
                    if interleave:
                        nc.sync.dma_start(out=xr[:, k, :], in_=xT[rows, :])
                for k in range(KT):
                    rows = slice(k * 128, (k + 1) * 128)
                    nc.sync.dma_start(out=wk_g[:, k, :], in_=wk[rows, c0:c0 + gw])
                    if interleave:
                        nc.sync.dma_start(out=ct[:, k, :], in_=condT[rows, :])
                for k in range(KT):
                    rows = slice(k * 128, (k + 1) * 128)
                    nc.sync.dma_start(out=wv_g[:, k, :], in_=wv[rows, c0:c0 + gw])
                return wq_g, wk_g, wv_g

            def alloc_group_qkv(g):
                qT_g = grp.tile([128, HPG, IQ], BF16, tag="qT_g", name=f"qT_{g}")
                kT_g = grp.tile([128, HPG, NK], BF16, tag="kT_g", name=f"kT_{g}")
                v_g = grp.tile([128, JT, HPG * DH], FP16, tag="v_g", name=f"v_{g}")
                return qT_g, kT_g, v_g

            def proj_gen(g, wq_g, wk_g, wv_g, qT_g, kT_g, v_g):
                """One yield per emitted instruction. Single-live PSUM acc."""
                gw = HPG * DH
                # Q projection: qT_g[:, mh, ih*512:] = Wq_slice^T @ x^T
                for ih in range(IH):
                    for mh in range(HPG):
                        acc = ps.tile([128, 512], F32, tag="pp", bufs=2,
                                      name=f"q{g}_{ih}_{mh}")
                        for k in range(KT):
                            nc.tensor.matmul(
                                acc,
                                wq_g[:, k, mh * DH:(mh + 1) * DH],
                                xr[:, k, ih * 512:(ih + 1) * 512],
                                start=(k == 0), stop=(k == KT - 1))
                            yield
                        nc.vector.tensor_copy(
                            qT_g[:, mh, ih * 512:(ih + 1) * 512], acc)
                        yield
                # K projection: kT_g[:, mh, jh*512:] = Wk_slice^T @ cond^T
                for jh in range(NK // 512):
                    for mh in range(HPG):
                        acc = ps.tile([128, 512], F32, tag="pp", bufs=2,
                                      name=f"k{g}_{jh}_{mh}")
                        for k in range(KT):
                            nc.tensor.matmul(
                                acc,
                                wk_g[:, k, mh * DH:(mh + 1) * DH],
                                ct[:, k, jh * 512:(jh + 1) * 512],
                                start=(k == 0), stop=(k == KT - 1))
                            yield
                        nc.vector.tensor_copy(
                            kT_g[:, mh, jh * 512:(jh + 1) * 512], acc)
                        yield
                # V projection: v_g[:, jt, :] = cond^T_jt^T @ Wv_slice
                for jt in range(JT):
                    acc = ps.tile([128, 512], F32, tag="pp", bufs=2,
                                  name=f"v{g}_{jt}")
                    for k in range(KT):
                        nc.tensor.matmul(
                            acc[:, :gw],
                            ct[:, k, jt * 128:(jt + 1) * 128],
                            wv_g[:, k, :],
                            start=(k == 0), stop=(k == KT - 1))
                        yield
                    nc.vector.tensor_copy(v_g[:, jt, :], acc[:, :gw])
                    yield

            def oproj_head_gen():
                """O-proj partial sums over k=0..OSPLIT-1 into SBUF."""
                for it in range(IQ // 128):
                    for nh in range(D // 512):
                        fo = ps.tile([128, 512], F32, tag="pp", bufs=2,
                                     name=f"oa_{it}_{nh}")
                        for k in range(OSPLIT):
                            nc.tensor.matmul(
                                fo,
                                attT[:, k, it * 128:(it + 1) * 128],
                                wo_r[:, k, nh * 512:(nh + 1) * 512],
                                start=(k == 0), stop=(k == OSPLIT - 1))
                            yield
                        nc.vector.tensor_copy(foa[:, it, nh, :], fo)
                        yield

            def drain(gen, n):
                for _ in range(n):
                    if gen is None:
                        return
                    try:
                        next(gen)
                    except StopIteration:
                        return

            def attention(g, qT_g, kT_g, v_g, fill):
                for hg in range(HPG):
                    h = g * HPG + hg
                    den_s = small.tile([128, IQ], FP16, tag="den_s",
                                       name=f"den_{h}")
                    avs = [ps.tile([128, 512], F32, tag="av", bufs=4,
                                   name=f"av_{h}_{i}") for i in range(IH)]
                    for jt in range(JT):
                        escs = []
                        for ih in range(IH):
                            sc = ps.tile([128, 512], F32, tag="sc", bufs=2,
                                         name=f"sc_{h}_{jt}_{ih}")
                            nc.tensor.matmul(
                                sc,
                                kT_g[:, hg, jt * 128:(jt + 1) * 128],
                                qT_g[:, hg, ih * 512:(ih + 1) * 512],
                                start=True, stop=True)
                            esc = expp.tile([128, 512], FP16, tag="esc")
                            nc.scalar.activation(esc, sc, EXP)
                            escs.append(esc)
                        drain(fill, 4)
                        for ih in range(IH):
                            nc.tensor.matmul(
                                avs[ih],
                                v_g[:, jt, hg * DH:(hg + 1) * DH],
                                escs[ih],
                                start=(jt == 0), stop=(jt == JT - 1))
                        for ih in range(IH):
                            sl = slice(ih * 512, (ih + 1) * 512)
                            if jt == 0:
                                nc.vector.tensor_copy(den_s[:, sl], escs[ih])
                            else:
                                nc.gpsimd.tensor_add(den_s[:, sl],
                                                     den_s[:, sl], escs[ih])
                        drain(fill, 4)
                    # denominator partition-reduce + reciprocal + normalize
                    for ih in range(IH):
                        sl = slice(ih * 512, (ih + 1) * 512)
                        den_bc = ps.tile([128, 512], F32, tag="sc", bufs=2,
                                         name=f"denbc_{h}_{ih}")
                        nc.tensor.matmul(den_bc, ones, den_s[:, sl],
                                         start=True, stop=True)
                        with tc.high_priority():
                            r_den = small.tile([128, 512], FP16, tag="rden",
                                               name=f"rd_{h}_{ih}")
                            nc.vector.reciprocal(r_den, den_bc)
                            nc.vector.tensor_mul(attT[:, h, sl], avs[ih], r_den)

            # ---- startup DMAs: group-0 weights interleaved with x/cond ----
            wq_g, wk_g, wv_g = load_group_weights(0, interleave=True)
            nc.gpsimd.memset(ones_f, 1.0)
            nc.vector.tensor_copy(ones, ones_f)

            qkv = alloc_group_qkv(0)
            for _ in proj_gen(0, wq_g, wk_g, wv_g, *qkv):
                pass

            for g in range(GROUPS):
                if g + 1 < GROUPS:
                    nw = load_group_weights(g + 1)
                    nqkv = alloc_group_qkv(g + 1)
                    fill = proj_gen(g + 1, *nw, *nqkv)
                else:
                    fill = oproj_head_gen()
                if g == 1:
                    for k in range(KT):
                        nc.sync.dma_start(out=wo_r[:, k, :],
                                          in_=wo[k * 128:(k + 1) * 128, :])
                    nc.sync.dma_start(out=bo_bc,
                                      in_=bo[:, :].to_broadcast((128, D)))
                attention(g, *qkv, fill)
                drain(fill, 10 ** 6)
                if g + 1 < GROUPS:
                    qkv = nqkv

            # ---- O-proj finish: k=OSPLIT..KT-1, add staged partial + bias ----
            for it in range(IQ // 128):
                for nh in range(D // 512):
                    fo = ps.tile([128, 512], F32, tag="pp", bufs=2,
                                 name=f"ob_{it}_{nh}")
                    for k in range(OSPLIT, KT):
                        nc.tensor.matmul(
                            fo,
                            attT[:, k, it * 128:(it + 1) * 128],
                            wo_r[:, k, nh * 512:(nh + 1) * 512],
                            start=(k == OSPLIT), stop=(k == KT - 1))
                    fo_sb = ostage.tile([128, 512], F32, tag="fo_sb")
                    nc.vector.tensor_add(fo_sb, fo, foa[:, it, nh, :])
                    nc.vector.tensor_add(fo_sb, fo_sb,
                                         bo_bc[:, nh * 512:(nh + 1) * 512])
                    nc.sync.dma_start(
                        out=out[it * 128:(it + 1) * 128,
                                nh * 512:(nh + 1) * 512],
                        in_=fo_sb)
    nc.finalize()
    return nc


_NC_CACHE = None


def _get_nc():
    global _NC_CACHE
    if _NC_CACHE is None:
        _NC_CACHE = build_nc()
    return _NC_CACHE


def make_in_maps(x, cond, Wq, Wk, Wv, Wo, bo):
    bf = ml_dtypes.bfloat16
    wq_s = np.ascontiguousarray((Wq * SCALE).astype(bf))
    wk_c = np.ascontiguousarray(Wk.astype(bf))
    wv_c = np.ascontiguousarray(Wv.astype(bf))
    wo_c = np.ascontiguousarray(Wo.astype(bf))
    bo_c = np.ascontiguousarray(bo, dtype=np.float32).reshape(1, D)
    in_maps = []
    for c in range(NCORES):
        b, ih = c // 2, c % 2
        in_maps.append({
            "xT": np.ascontiguousarray(
                x[b, ih * IQ:(ih + 1) * IQ, :].T.astype(bf)),
            "condT": np.ascontiguousarray(cond[b].T.astype(bf)),
            "wq": wq_s, "wk": wk_c, "wv": wv_c, "wo": wo_c, "bo": bo_c,
        })
    return in_maps


def kernel(x, cond, Wq, Wk, Wv, Wo, bo, _trace=False, _trace_kwargs=None):
    x = np.asarray(x, dtype=np.float32)
    cond = np.asarray(cond, dtype=np.float32)
    nc = _get_nc()
    in_maps = make_in_maps(x, cond,
                           np.asarray(Wq, np.float32), np.asarray(Wk, np.float32),
                           np.asarray(Wv, np.float32), np.asarray(Wo, np.float32),
                           np.asarray(bo, np.float32))
    kw = {}
    if _trace:
        kw = {"trace": True, "trace_kwargs": _trace_kwargs or {}}
    res = run_bass_kernel_spmd(nc, in_maps, list(range(NCORES)), **kw)
    out = np.empty((B, NQ, D), dtype=np.float32)
    for c in range(NCORES):
        b, ih = c // 2, c % 2
        out[b, ih * IQ:(ih + 1) * IQ, :] = res.results[c]["out"]
    if _trace:
        return out, res
    return out


if __name__ == "__main__":
    # quick numeric self-check against numpy (no jax needed)
    rng = np.random.default_rng(0)
    s = 0.02
    x = rng.standard_normal((B, NQ, D), dtype=np.float32)
    cond = rng.standard_normal((B, NK, D), dtype=np.float32)
    Wq = (rng.standard_normal((D, D), dtype=np.float32) * s)
    Wk = (rng.standard_normal((D, D), dtype=np.float32) * s)
    Wv = (rng.standard_normal((D, D), dtype=np.float32) * s)
    Wo = (rng.standard_normal((D, D), dtype=np.float32) * s)
    bo = (rng.standard_normal((D,), dtype=np.float32) * s)

    def ref_np(x, cond):
        q = (x @ Wq).reshape(B, NQ, H, DH).transpose(0, 2, 1, 3)
        k = (cond @ Wk).reshape(B, NK, H, DH).transpose(0, 2, 1, 3)
        v = (cond @ Wv).reshape(B, NK, H, DH).transpose(0, 2, 1, 3)
        sim = np.einsum('bhid,bhjd->bhij', q, k) * SCALE
        sim = sim - sim.max(axis=-1, keepdims=True)
        a = np.exp(sim)
        a = a / a.sum(axis=-1, keepdims=True)
        o = np.einsum('bhij,bhjd->bhid', a, v)
        o = o.transpose(0, 2, 1, 3).reshape(B, NQ, D)
        return o @ Wo + bo

    import time
    t0 = time.time()
    got = kernel(x=x, cond=cond, Wq=Wq, Wk=Wk, Wv=Wv, Wo=Wo, bo=bo)
    print(f"kernel run {time.time()-t0:.1f}s")
    exp = ref_np(x.astype(np.float64), cond.astype(np.float64))
    err = np.abs(got - exp)
    rel = np.linalg.norm(got - exp) / np.linalg.norm(exp)
    print(f"rel_l2={rel:.3e} absmax_rel={err.max()/np.abs(exp).max():.3e}")


# revision 11
# speedup vs baseline: 1.1106x; 1.1106x over previous
"""CrossAttention TRN2 kernel (v3).

Problem (hardcoded shapes):
  x    [4, 2048, 1024], cond [4, 2048, 1024]
  Wq/Wk/Wv [1024, 1024], Wo [1024, 1024], bo [1024]
  out = softmax((x@Wq) 8 heads of 128 @ (cond@Wk)^T * 0.125) @ (cond@Wv) @ Wo + bo

Sharding: 8 cores = (batch b in 0..3) x (query-half ih in 0..1).
Each core: 1024 query rows of one batch, all 8 heads; K/V projection for
that batch replicated across the 2 cores sharing it. No collectives.

v3 design:
- Matmul inputs bf16 (x, cond, weights; cast on host); exp/V/denominator in
  fp16 (16-bit DVE/pool rate, no mixed-dtype ops); PSUM f32.
- x^T and cond^T fully resident in SBUF.
- Softmax denominator: gpsimd accumulates fp16 exp tiles (SBUF only), a
  ones-stationary fp16 matmul does the partition reduction into PSUM per
  (head, ih), DVE reciprocal (fp16 out), DVE high-priority normalize mul.
- Head group g's attention interleaves group g+1's projection matmuls into
  the tensor queue so the PE never waits on the scalar engine's exp.
- O-projection split-k: k=0..5 (heads 0..5) interleaved into group-3
  attention, k=6..7 + combine + bias at the end.
- av PSUM tiles bufs=4 and single-live projection accumulators, so head
  boundaries don't stall on PSUM bank reuse.
"""
import numpy as np
import ml_dtypes

import concourse.bass as bass
import concourse.bacc as bacc
import concourse.tile as tile
from concourse import bass_isa, mybir
from concourse.bass_utils import run_bass_kernel_spmd

F32 = mybir.dt.float32
F32R = mybir.dt.float32r
BF16 = mybir.dt.bfloat16
FP16 = mybir.dt.float16
EXP = mybir.ActivationFunctionType.Exp

B, NQ, NK, D = 4, 2048, 2048, 1024   # D = query_dim = cond_dim = inner_dim
H, DH = 8, 128                        # heads, per-head dim
SCALE = 64 ** -0.5                    # reference uses dim_head=64 for the scale
NCORES = 8
IQ = NQ // 2                          # query rows per core (1024)
KT = D // 128                         # contraction tiles (8)
GROUPS, HPG = 4, 2                    # head groups of 2 heads
JT = NK // 128                        # key tiles (16)
IH = IQ // 512                        # 512-wide query chunks (2)
OSPLIT = 6                            # O-proj k tiles interleaved into att(3)


def build_nc():
    nc = bacc.Bacc()
    xT = nc.declare_dram_parameter("xT", [D, IQ], BF16, isOutput=False)
    condT = nc.declare_dram_parameter("condT", [D, NK], BF16, isOutput=False)
    wq = nc.declare_dram_parameter("wq", [D, D], BF16, isOutput=False)
    wk = nc.declare_dram_parameter("wk", [D, D], BF16, isOutput=False)
    wv = nc.declare_dram_parameter("wv", [D, D], BF16, isOutput=False)
    wo = nc.declare_dram_parameter("wo", [D, D], BF16, isOutput=False)
    bo = nc.declare_dram_parameter("bo", [1, D], F32, isOutput=False)
    out = nc.declare_dram_parameter("out", [IQ, D], F32, isOutput=True)

    with tile.TileContext(nc) as tc:
        with (
            nc.allow_low_precision(reason="bf16/fp16 matmul operands intended"),
            tc.tile_pool(name="const", bufs=1) as const,
            tc.tile_pool(name="big", bufs=1) as big,
            tc.tile_pool(name="grp", bufs=2) as grp,
            tc.tile_pool(name="expp", bufs=4) as expp,
            tc.tile_pool(name="small", bufs=2) as small,
            tc.tile_pool(name="ostage", bufs=2) as ostage,
            tc.tile_pool(name="oacc", bufs=1) as oacc,
            tc.tile_pool(name="ps", bufs=1, space="PSUM") as ps,
        ):
            bo_bc = const.tile([128, D], F32)
            ones = const.tile([128, 128], FP16)
            ones_f = const.tile([128, 128], F32)

            xr = big.tile([128, KT, IQ], BF16)    # resident x^T
            ct = big.tile([128, KT, NK], BF16)    # resident cond^T
            attT = big.tile([128, H, IQ], BF16)   # attention out, [dh, head, i]
            wo_r = big.tile([128, KT, D], BF16)
            # O-proj partial sums (k=0..OSPLIT-1), staged in SBUF
            foa = oacc.tile([128, IQ // 128, D // 512, 512], F32)

            def load_group_weights(g, interleave=False):
                c0 = g * HPG * DH
                gw = HPG * DH
                wq_g = grp.tile([128, KT, gw], BF16, tag="wq_g", name=f"wq_{g}")
                wk_g = grp.tile([128, KT, gw], BF16, tag="wk_g", name=f"wk_{g}")
                wv_g = grp.tile([128, KT, gw], BF16, tag="wv_g", name=f"wv_{g}")
                for k in range(KT):
                    rows = slice(k * 128, (k + 1) * 128)
                    nc.sync.dma_start(out=wq_g[:, k, :], in_=wq[rows, c0:c0 + gw])
                    if interleave:
                        nc.sync.dma_start(out=xr[:, k, :], in_=xT[rows, :])
                for k in range(KT):
                    rows = slice(k * 128, (k + 1) * 128)
                    nc.sync.dma_start(out=wk_g[:, k, :], in_=wk[rows, c0:c0 + gw])
                    if interleave:
                        nc.sync.dma_start(out=ct[:, k, :], in_=condT[rows, :])
                for k in range(KT):
                    rows = slice(k * 128, (k + 1) * 128)
                    nc.sync.dma_start(out=wv_g[:, k, :], in_=wv[rows, c0:c0 + gw])
                return wq_g, wk_g, wv_g

            def alloc_group_qkv(g):
                qT_g = grp.tile([128, HPG, IQ], BF16, tag="qT_g", name=f"qT_{g}")
                kT_g = grp.tile([128, HPG, NK], BF16, tag="kT_g", name=f"kT_{g}")
                v_g = grp.tile([128, JT, HPG * DH], FP16, tag="v_g", name=f"v_{g}")
                return qT_g, kT_g, v_g

            def proj_gen(g, wq_g, wk_g, wv_g, qT_g, kT_g, v_g):
                """One yield per emitted instruction. Single-live PSUM acc."""
                gw = HPG * DH
                # Q projection: qT_g[:, mh, ih*512:] = Wq_slice^T @ x^T
                for ih in range(IH):
                    for mh in range(HPG):
                        acc = ps.tile([128, 512], F32, tag="pp", bufs=2,
                                      name=f"q{g}_{ih}_{mh}")
                        for k in range(KT):
                            nc.tensor.matmul(
                                acc,
                                wq_g[:, k, mh * DH:(mh + 1) * DH],
                                xr[:, k, ih * 512:(ih + 1) * 512],
                                start=(k == 0), stop=(k == KT - 1))
                            yield
                        nc.vector.tensor_copy(
                            qT_g[:, mh, ih * 512:(ih + 1) * 512], acc)
                        yield
                # K projection: kT_g[:, mh, jh*512:] = Wk_slice^T @ cond^T
                for jh in range(NK // 512):
                    for mh in range(HPG):
                        acc = ps.tile([128, 512], F32, tag="pp", bufs=2,
                                      name=f"k{g}_{jh}_{mh}")
                        for k in range(KT):
                            nc.tensor.matmul(
                                acc,
                                wk_g[:, k, mh * DH:(mh + 1) * DH],
                                ct[:, k, jh * 512:(jh + 1) * 512],
                                start=(k == 0), stop=(k == KT - 1))
                            yield
                        nc.vector.tensor_copy(
                            kT_g[:, mh, jh * 512:(jh + 1) * 512], acc)
                        yield
                # V projection: v_g[:, jt, :] = cond^T_jt^T @ Wv_slice
                for jt in range(JT):
                    acc = ps.tile([128, 512], F32, tag="pp", bufs=2,
                                  name=f"v{g}_{jt}")
                    for k in range(KT):
                        nc.tensor.matmul(
                            acc[:, :gw],
                            ct[:, k, jt * 128:(jt + 1) * 128],
                            wv_g[:, k, :],
                            start=(k == 0), stop=(k == KT - 1))
                        yield
                    nc.vector.tensor_copy(v_g[:, jt, :], acc[:, :gw])
                    yield

            def oproj_head_gen():
                """O-proj partial sums over k=0..OSPLIT-1 into SBUF."""
                for it in range(IQ // 128):
                    for nh in range(D // 512):
                        fo = ps.tile([128, 512], F32, tag="pp", bufs=2,
                                     name=f"oa_{it}_{nh}")
                        for k in range(OSPLIT):
                            nc.tensor.matmul(
                                fo,
                                attT[:, k, it * 128:(it + 1) * 128],
                                wo_r[:, k, nh * 512:(nh + 1) * 512],
                                start=(k == 0), stop=(k == OSPLIT - 1))
                            yield
                        nc.vector.tensor_add(foa[:, it, nh, :], fo,
                                             bo_bc[:, nh * 512:(nh + 1) * 512])
                        yield

            def drain(gen, n):
                for _ in range(n):
                    if gen is None:
                        return
                    try:
                        next(gen)
                    except StopIteration:
                        return

            def den_path(h, den_s, avs):
                """Partition-reduce + reciprocal + normalize for head h.
                Returned as a closure so it can be emitted a couple of jt
                iterations into the NEXT head (the in-order tensor queue
                then never waits on the denominator accumulators)."""
                def emit():
                    for ih in range(IH):
                        sl = slice(ih * 512, (ih + 1) * 512)
                        den_bc = ps.tile([128, 512], F32, tag="sc", bufs=2,
                                         name=f"denbc_{h}_{ih}")
                        nc.tensor.matmul(den_bc, ones, den_s[:, sl],
                                         start=True, stop=True)
                        with tc.high_priority():
                            r_den = small.tile([128, 512], FP16, tag="rden",
                                               name=f"rd_{h}_{ih}")
                            nc.vector.reciprocal(r_den, den_bc)
                            nc.vector.tensor_mul(attT[:, h, sl], avs[ih], r_den)
                return emit

            def attention(g, qT_g, kT_g, v_g, fill, pending, flush_early=False):
                # at group 3 the fill reads attT heads 0..5, so head 5's
                # deferred den-path must be emitted before any fill drains
                if flush_early and pending is not None:
                    pending()
                    pending = None
                for hg in range(HPG):
                    h = g * HPG + hg
                    den_s = small.tile([128, IQ], FP16, tag="den_s",
                                       name=f"den_{h}")
                    avs = [ps.tile([128, 512], F32, tag="av", bufs=4,
                                   name=f"av_{h}_{i}") for i in range(IH)]
                    for jt in range(JT):
                        escs = []
                        for ih in range(IH):
                            sc = ps.tile([128, 512], F32, tag="sc", bufs=2,
                                         name=f"sc_{h}_{jt}_{ih}")
                            nc.tensor.matmul(
                                sc,
                                kT_g[:, hg, jt * 128:(jt + 1) * 128],
                                qT_g[:, hg, ih * 512:(ih + 1) * 512],
                                start=True, stop=True)
                            esc = expp.tile([128, 512], FP16, tag="esc")
                            nc.scalar.activation(esc, sc, EXP)
                            escs.append(esc)
                        drain(fill, 4)
                        for ih in range(IH):
                            nc.tensor.matmul(
                                avs[ih],
                                v_g[:, jt, hg * DH:(hg + 1) * DH],
                                escs[ih],
                                start=(jt == 0), stop=(jt == JT - 1))
                        # denominator accumulation: ih0 on gpsimd, ih1 on DVE
                        if jt == 0:
                            nc.gpsimd.tensor_copy(den_s[:, :512], escs[0])
                            nc.vector.tensor_copy(den_s[:, 512:], escs[1])
                        else:
                            nc.gpsimd.tensor_add(den_s[:, :512],
                                                 den_s[:, :512], escs[0])
                            nc.vector.tensor_add(den_s[:, 512:],
                                                 den_s[:, 512:], escs[1])
                        if jt == 2 and pending is not None:
                            pending()
                            pending = None
                        drain(fill, 4)
                    pending = den_path(h, den_s, avs)
                return pending

            # ---- startup DMAs: group-0 weights interleaved with x/cond ----
            wq_g, wk_g, wv_g = load_group_weights(0, interleave=True)
            nc.gpsimd.memset(ones_f, 1.0)
            nc.vector.tensor_copy(ones, ones_f)

            qkv = alloc_group_qkv(0)
            for _ in proj_gen(0, wq_g, wk_g, wv_g, *qkv):
                pass

            pending = None
            for g in range(GROUPS):
                if g + 1 < GROUPS:
                    nw = load_group_weights(g + 1)
                    nqkv = alloc_group_qkv(g + 1)
                    fill = proj_gen(g + 1, *nw, *nqkv)
                else:
                    fill = oproj_head_gen()
                if g == 1:
                    for k in range(KT):
                        nc.sync.dma_start(out=wo_r[:, k, :],
                                          in_=wo[k * 128:(k + 1) * 128, :])
                    nc.sync.dma_start(out=bo_bc,
                                      in_=bo[:, :].to_broadcast((128, D)))
                pending = attention(g, *qkv, fill, pending,
                                    flush_early=(g == GROUPS - 1))
                drain(fill, 10 ** 6)
                if g + 1 < GROUPS:
                    qkv = nqkv
            pending()

            # ---- O-proj finish: k=OSPLIT..KT-1, add staged partial + bias ----
            for it in range(IQ // 128):
                for nh in range(D // 512):
                    fo = ps.tile([128, 512], F32, tag="pp", bufs=2,
                                 name=f"ob_{it}_{nh}")
                    for k in range(OSPLIT, KT):
                        nc.tensor.matmul(
                            fo,
                            attT[:, k, it * 128:(it + 1) * 128],
                            wo_r[:, k, nh * 512:(nh + 1) * 512],
                            start=(k == OSPLIT), stop=(k == KT - 1))
                    fo_sb = ostage.tile([128, 512], F32, tag="fo_sb")
                    nc.vector.tensor_add(fo_sb, fo, foa[:, it, nh, :])
                    nc.sync.dma_start(
                        out=out[it * 128:(it + 1) * 128,
                                nh * 512:(nh + 1) * 512],
                        in_=fo_sb)
    nc.finalize()
    return nc


_NC_CACHE = None


def _get_nc():
    global _NC_CACHE
    if _NC_CACHE is None:
        _NC_CACHE = build_nc()
    return _NC_CACHE


def make_in_maps(x, cond, Wq, Wk, Wv, Wo, bo):
    bf = ml_dtypes.bfloat16
    wq_s = np.ascontiguousarray((Wq * SCALE).astype(bf))
    wk_c = np.ascontiguousarray(Wk.astype(bf))
    wv_c = np.ascontiguousarray(Wv.astype(bf))
    wo_c = np.ascontiguousarray(Wo.astype(bf))
    bo_c = np.ascontiguousarray(bo, dtype=np.float32).reshape(1, D)
    in_maps = []
    for c in range(NCORES):
        b, ih = c // 2, c % 2
        in_maps.append({
            "xT": np.ascontiguousarray(
                x[b, ih * IQ:(ih + 1) * IQ, :].T.astype(bf)),
            "condT": np.ascontiguousarray(cond[b].T.astype(bf)),
            "wq": wq_s, "wk": wk_c, "wv": wv_c, "wo": wo_c, "bo": bo_c,
        })
    return in_maps


def kernel(x, cond, Wq, Wk, Wv, Wo, bo, _trace=False, _trace_kwargs=None):
    x = np.asarray(x, dtype=np.float32)
    cond = np.asarray(cond, dtype=np.float32)
    nc = _get_nc()
    in_maps = make_in_maps(x, cond,
                           np.asarray(Wq, np.float32), np.asarray(Wk, np.float32),
                           np.asarray(Wv, np.float32), np.asarray(Wo, np.float32),
                           np.asarray(bo, np.float32))
    kw = {}
    if _trace:
        kw = {"trace": True, "trace_kwargs": _trace_kwargs or {}}
    res = run_bass_kernel_spmd(nc, in_maps, list(range(NCORES)), **kw)
    out = np.empty((B, NQ, D), dtype=np.float32)
    for c in range(NCORES):
        b, ih = c // 2, c % 2
        out[b, ih * IQ:(ih + 1) * IQ, :] = res.results[c]["out"]
    if _trace:
        return out, res
    return out


if __name__ == "__main__":
    # quick numeric self-check against numpy (no jax needed)
    rng = np.random.default_rng(0)
    s = 0.02
    x = rng.standard_normal((B, NQ, D), dtype=np.float32)
    cond = rng.standard_normal((B, NK, D), dtype=np.float32)
    Wq = (rng.standard_normal((D, D), dtype=np.float32) * s)
    Wk = (rng.standard_normal((D, D), dtype=np.float32) * s)
    Wv = (rng.standard_normal((D, D), dtype=np.float32) * s)
    Wo = (rng.standard_normal((D, D), dtype=np.float32) * s)
    bo = (rng.standard_normal((D,), dtype=np.float32) * s)

    def ref_np(x, cond):
        q = (x @ Wq).reshape(B, NQ, H, DH).transpose(0, 2, 1, 3)
        k = (cond @ Wk).reshape(B, NK, H, DH).transpose(0, 2, 1, 3)
        v = (cond @ Wv).reshape(B, NK, H, DH).transpose(0, 2, 1, 3)
        sim = np.einsum('bhid,bhjd->bhij', q, k) * SCALE
        sim = sim - sim.max(axis=-1, keepdims=True)
        a = np.exp(sim)
        a = a / a.sum(axis=-1, keepdims=True)
        o = np.einsum('bhij,bhjd->bhid', a, v)
        o = o.transpose(0, 2, 1, 3).reshape(B, NQ, D)
        return o @ Wo + bo

    import time
    t0 = time.time()
    got = kernel(x=x, cond=cond, Wq=Wq, Wk=Wk, Wv=Wv, Wo=Wo, bo=bo)
    print(f"kernel run {time.time()-t0:.1f}s")
    exp = ref_np(x.astype(np.float64), cond.astype(np.float64))
    err = np.abs(got - exp)
    rel = np.linalg.norm(got - exp) / np.linalg.norm(exp)
    print(f"rel_l2={rel:.3e} absmax_rel={err.max()/np.abs(exp).max():.3e}")
